# revision 1
# baseline (speedup 1.0000x reference)
"""Trainium2 Bass kernel for nn_EqStftPBC (STFT perturbation-based compensation).

Per (batch b, mode m):
  X = STFT(x); C_n2 = X*conj(roll(X,n2)) + prev-frame; U_n2 = circulant(w[:,n2]);
  V_n2 = U_n2 * roll(X,n2); delta = sum_n2 V_n2; out = x + ISTFT(delta)*P (+bias)

8 cores = (b x m x n2-half), uniform SPMD program; per-core variation only in
input data (permutation stack S, circulant stack M).  Device layout: [freq(80)
partitions, time free].  STFT fp32, rest bf16 (PSUM fp32).
"""

import numpy as np
from ml_dtypes import bfloat16

import concourse.bass as bass
import concourse.bacc as bacc
import concourse.mybir as mybir
import concourse.tile as tile

F = 80
T = 51
TP = 52          # padded slot stride
HOP = 40
L = 2080
NJ = 20
NCH = 2
CHJ = NJ // NCH
PBK = 5          # stage-1/R psum outputs per bank
GJ = 5           # j per merged G-matmul (N = GJ*102 <= 512)
FP32 = mybir.dt.float32
BF16 = mybir.dt.bfloat16

N2_LISTS = [list(range(19, -1, -1)), list(range(-1, -21, -1))]


def _dft_consts():
    j = np.arange(F)
    W = np.exp(-2j * np.pi * np.outer(j, j) / F)
    G = np.exp(+2j * np.pi * np.outer(j, j) / F) / F
    return W, G


def build_program(debug=False):
    nc = bacc.Bacc("TRN2", target_bir_lowering=False, debug=debug)

    # xf = [fiN | fr | fi] frames, pre-framed on host (pure reshape)
    xf = nc.dram_tensor("xf", [F, 3 * T], BF16, kind="ExternalInput")
    fr_c = nc.dram_tensor("fr_c", [F, 2 * F], BF16, kind="ExternalInput")
    gr_c = nc.dram_tensor("gr_c", [F, 2 * F], BF16, kind="ExternalInput")
    smat = nc.dram_tensor("smat", [F, NJ * F], BF16, kind="ExternalInput")
    mst = nc.dram_tensor("mst", [F, NJ * 2 * F], BF16, kind="ExternalInput")
    svec = nc.dram_tensor("svec", [HOP, 52], FP32, kind="ExternalInput")
    yv = nc.dram_tensor("yv", [HOP, 2 * 52], FP32, kind="ExternalOutput")

    with tile.TileContext(nc) as tc:
        with (
            tc.tile_pool(name="const", bufs=1) as cpool,
            tc.tile_pool(name="work", bufs=1) as wpool,
            tc.tile_pool(name="ps_x", bufs=1, space="PSUM") as ps_x,
            tc.tile_pool(name="ps_r", bufs=2, space="PSUM") as ps_r,
            tc.tile_pool(name="ps_u", bufs=2, space="PSUM") as ps_u,
            tc.tile_pool(name="ps_d", bufs=1, space="PSUM") as ps_d,
        ):
            frm = wpool.tile([F, 3 * T], BF16, tag="frm")
            nc.sync.dma_start(frm[:, :], xf[:, :])
            Fc = cpool.tile([F, 2 * F], BF16, tag="Fc")
            nc.sync.dma_start(Fc[:, :], fr_c[:, :])
            Ssb = cpool.tile([F, NJ * F], BF16, tag="Ssb")
            for q in range(NJ // PBK):
                nc.sync.dma_start(Ssb[:, q * PBK * F:(q + 1) * PBK * F],
                                  smat[:, q * PBK * F:(q + 1) * PBK * F])
            Msb = cpool.tile([F, NJ * 2 * F], BF16, tag="Msb")
            for c in range(NCH):
                nc.gpsimd.dma_start(Msb[:, c * CHJ * 2 * F:(c + 1) * CHJ * 2 * F],
                                    mst[:, c * CHJ * 2 * F:(c + 1) * CHJ * 2 * F])
            Gc = cpool.tile([F, 2 * F], BF16, tag="Gc")
            nc.gpsimd.dma_start(Gc[:, :], gr_c[:, :])
            sv = cpool.tile([HOP, 52], FP32, tag="sv")
            nc.gpsimd.dma_start(sv[:, :], svec[:, :])

            # ---- STFT (fp32) -> X bf16 [Xr(52) | Xi(52)] ----
            Xp = ps_x.tile([F, 2 * T], FP32, tag="Xp")
            nc.tensor.matmul(Xp[:, :], Fc[:, 0:F], frm[:, T:3 * T], start=True, stop=False)
            nc.tensor.matmul(Xp[:, :], Fc[:, F:2 * F], frm[:, 0:2 * T], start=False, stop=True)
            Xsb = wpool.tile([F, 2 * TP], BF16, tag="Xsb")
            Xsv = Xsb[:, :].rearrange("p (c t) -> p c t", c=2)
            nc.scalar.activation(Xsv[:, :, 0:T],
                                 Xp[:, :].rearrange("p (c t) -> p c t", c=2),
                                 mybir.ActivationFunctionType.Copy)
            Xrhs = bass.AP(tensor=Xsb[:, :].tensor, offset=Xsb[:, :].offset,
                           ap=[[2 * TP, F], [TP, 2], [1, T]])

            # plane-major per-chunk stacks: R/U = [r-block | i-block], blocks CHJ*TP
            # C/V = [negi-block | r-block | i-block]
            BL = CHJ * TP
            Rsb, Csb, Usb, Vsb = [], [], [], []
            for c in range(NCH):
                Rsb.append(wpool.tile([F, 2 * BL], BF16, tag=f"Rsb{c}", name=f"Rsb{c}"))
                Csb.append(wpool.tile([F, 3 * BL], BF16, tag=f"Csb{c}", name=f"Csb{c}"))
                Usb.append(wpool.tile([F, 2 * BL], BF16, tag=f"Usb{c}", name=f"Usb{c}"))
                Vsb.append(wpool.tile([F, 3 * BL], BF16, tag=f"Vsb{c}", name=f"Vsb{c}"))
            sA = wpool.tile([F, BL], BF16, tag="sA")
            sB = wpool.tile([F, BL], BF16, tag="sB")
            sC = wpool.tile([F, BL], BF16, tag="sC")
            sD = wpool.tile([F, BL], BF16, tag="sD")
            sPR = wpool.tile([F, BL], BF16, tag="sPR")
            sPI = wpool.tile([F, BL], BF16, tag="sPI")

            Dps = [ps_d.tile([F, GJ * 2 * T], FP32, tag=f"Dp{c}", name=f"Dp{c}")
                   for c in range(NCH)]  # per-chunk accumulated [dr|di] pairs

            TT = nc.vector.tensor_tensor
            TG = nc.gpsimd.tensor_tensor
            MUL = mybir.AluOpType.mult
            ADD = mybir.AluOpType.add
            SUB = mybir.AluOpType.subtract
            CPY = mybir.ActivationFunctionType.Copy

            Dcs = []
            for c in range(NCH):
                Rc, Cc, Uc, Vc = Rsb[c], Csb[c], Usb[c], Vsb[c]
                # ---- R: permutation matmuls, PBK per bank, plane-major evict ----
                for bk in range(CHJ // PBK):
                    Rp = ps_r.tile([F, PBK * 2 * T], FP32, tag="Rp")
                    for s in range(PBK):
                        j = c * CHJ + bk * PBK + s
                        nc.tensor.matmul(Rp[:, s * 2 * T:(s + 1) * 2 * T],
                                         Ssb[:, j * F:(j + 1) * F],
                                         Xrhs, start=True, stop=True)
                    # psum [s, c2, t] -> Rsb [c2-block, (bk*PBK+s)*TP + t]
                    dst = bass.AP(tensor=Rc[:, :].tensor,
                                  offset=Rc[:, :].offset + bk * PBK * TP,
                                  ap=[[2 * BL, F], [TP, PBK], [BL, 2], [1, T]])
                    nc.scalar.activation(
                        dst, Rp[:, :].rearrange("p (s c2 t) -> p s c2 t", s=PBK, c2=2),
                        CPY)

                Rrf = Rc[:, 0:BL]
                Rif = Rc[:, BL:2 * BL]
                vPR = sPR[:, :].rearrange("p (j t) -> p j t", j=CHJ)
                vPI = sPI[:, :].rearrange("p (j t) -> p j t", j=CHJ)

                # ---- C_pre = X * conj(R)  (flat 2D ops; Xt = tiled X copies) ----
                if c == 0:
                    Xtr = wpool.tile([F, BL], BF16, tag="Xtr")
                    Xti = wpool.tile([F, BL], BF16, tag="Xti")
                    nc.scalar.activation(
                        Xtr[:, :].rearrange("p (j t) -> p j t", j=CHJ),
                        Xsb[:, None, 0:TP].to_broadcast([F, CHJ, TP]), CPY)
                    nc.scalar.activation(
                        Xti[:, :].rearrange("p (j t) -> p j t", j=CHJ),
                        Xsb[:, None, TP:2 * TP].to_broadcast([F, CHJ, TP]), CPY)
                TT(sA[:, :], Xtr[:, :], Rrf, MUL)
                TT(sB[:, :], Xti[:, :], Rif, MUL)
                TT(sPR[:, :], sA[:, :], sB[:, :], ADD)
                TG(sC[:, :], Xti[:, :], Rrf, MUL)
                TG(sD[:, :], Xtr[:, :], Rif, MUL)
                TG(sPI[:, :], sC[:, :], sD[:, :], SUB)

                # ---- C = C_pre + roll_t;  blocks [CiN | Cr | Ci] ----
                CrB = Cc[:, BL:2 * BL].rearrange("p (j t) -> p j t", j=CHJ)
                CiB = Cc[:, 2 * BL:3 * BL].rearrange("p (j t) -> p j t", j=CHJ)
                TT(CrB[:, :, 1:T], vPR[:, :, 1:T], vPR[:, :, 0:T - 1], ADD)
                TT(CrB[:, :, 0:1], vPR[:, :, 0:1], vPR[:, :, T - 1:T], ADD)
                TG(CiB[:, :, 1:T], vPI[:, :, 1:T], vPI[:, :, 0:T - 1], ADD)
                TG(CiB[:, :, 0:1], vPI[:, :, 0:1], vPI[:, :, T - 1:T], ADD)
                nc.scalar.activation(Cc[:, 0:BL], Cc[:, 2 * BL:3 * BL], CPY, scale=-1.0)

                # ---- stage-1: U_j = Mr.T@[Cr|Ci] + Mi.T@[CiN|Cr] ----
                for bk in range(CHJ // PBK):
                    Up = ps_u.tile([F, PBK * 2 * T], FP32, tag="Up")
                    for s in range(PBK):
                        jj = bk * PBK + s
                        j = c * CHJ + jj
                        rhs1 = bass.AP(tensor=Cc[:, :].tensor,
                                       offset=Cc[:, :].offset + BL + jj * TP,
                                       ap=[[3 * BL, F], [BL, 2], [1, T]])
                        rhs2 = bass.AP(tensor=Cc[:, :].tensor,
                                       offset=Cc[:, :].offset + jj * TP,
                                       ap=[[3 * BL, F], [BL, 2], [1, T]])
                        nc.tensor.matmul(Up[:, s * 2 * T:(s + 1) * 2 * T],
                                         Msb[:, (2 * j) * F:(2 * j + 1) * F],
                                         rhs1, start=True, stop=False)
                        nc.tensor.matmul(Up[:, s * 2 * T:(s + 1) * 2 * T],
                                         Msb[:, (2 * j + 1) * F:(2 * j + 2) * F],
                                         rhs2, start=False, stop=True)
                    dst = bass.AP(tensor=Uc[:, :].tensor,
                                  offset=Uc[:, :].offset + bk * PBK * TP,
                                  ap=[[2 * BL, F], [TP, PBK], [BL, 2], [1, T]])
                    nc.scalar.activation(
                        dst, Up[:, :].rearrange("p (s c2 t) -> p s c2 t", s=PBK, c2=2),
                        CPY)

                # ---- stage-2: V = U * R;  blocks [ViN | Vr | Vi]  (flat 2D) ----
                Urf = Uc[:, 0:BL]
                Uif = Uc[:, BL:2 * BL]
                TT(sA[:, :], Urf, Rrf, MUL)
                TT(sB[:, :], Uif, Rif, MUL)
                TT(Vc[:, BL:2 * BL], sA[:, :], sB[:, :], SUB)
                TG(sC[:, :], Urf, Rif, MUL)
                TG(sD[:, :], Uif, Rrf, MUL)
                TG(Vc[:, 2 * BL:3 * BL], sC[:, :], sD[:, :], ADD)
                nc.scalar.activation(Vc[:, 0:BL], Vc[:, 2 * BL:3 * BL], CPY, scale=-1.0)

                # ---- merged G-matmuls: accumulate into 5 [dr|di] pairs ----
                for gpass in range(2):
                    for h in range(CHJ // GJ):
                        base = (BL if gpass == 0 else 0) + h * GJ * TP
                        rhs = bass.AP(tensor=Vc[:, :].tensor,
                                      offset=Vc[:, :].offset + base,
                                      ap=[[3 * BL, F], [TP, GJ], [BL, 2], [1, T]])
                        nc.tensor.matmul(
                            Dps[c][:, :].rearrange("p (s c2 t) -> p s c2 t", s=GJ, c2=2),
                            Gc[:, gpass * F:(gpass + 1) * F], rhs,
                            start=(gpass == 0 and h == 0),
                            stop=(gpass == 1 and h == CHJ // GJ - 1))

                # per-chunk partial reduce (overlaps next chunk): 5 pairs -> 1
                D5 = wpool.tile([F, GJ * 2 * T], FP32, tag=f"D5{c}", name=f"D5{c}")
                nc.scalar.activation(D5[:, :], Dps[c][:, :], CPY)
                tE = wpool.tile([F, 4 * T], FP32, tag=f"tE{c}", name=f"tE{c}")
                TT(tE[:, :], D5[:, 0:4 * T], D5[:, 4 * T:8 * T], ADD)
                tF = wpool.tile([F, 2 * T], FP32, tag=f"tF{c}", name=f"tF{c}")
                TT(tF[:, :], tE[:, 0:2 * T], tE[:, 2 * T:4 * T], ADD)
                Dcs.append((tF, D5))

            # ---- final cross-chunk reduce + overlap-add + scale (fp32) ----
            tG = wpool.tile([F, 2 * T], FP32, tag="tG")
            TT(tG[:, :], Dcs[0][0][:, :], Dcs[1][0][:, :], ADD)
            tH = wpool.tile([F, 2 * T], FP32, tag="tH")
            TT(tH[:, :], Dcs[0][1][:, 8 * T:10 * T], Dcs[1][1][:, 8 * T:10 * T], ADD)
            Dsb = wpool.tile([F, 2 * T], FP32, tag="Dsb")
            TT(Dsb[:, :], tG[:, :], tH[:, :], ADD)

            S2 = wpool.tile([HOP, 2 * T], FP32, tag="S2")
            nc.sync.dma_start(S2[:, :], Dsb[HOP:F, :])
            Y = wpool.tile([HOP, 2 * 52], FP32, tag="Y")
            S1v = Dsb[0:HOP, :].rearrange("p (c t) -> p c t", c=2)
            S2v = S2[:, :].rearrange("p (c t) -> p c t", c=2)
            Yv = Y[:, :].rearrange("p (c t) -> p c t", c=2)
            TT(Yv[:, :, 1:T], S1v[:, :, 1:T], S2v[:, :, 0:T - 1], ADD)
            nc.scalar.activation(Yv[:, :, 0:1], S1v[:, :, 0:1], CPY)
            nc.scalar.activation(Yv[:, :, T:52], S2v[:, :, T - 1:T], CPY)
            TT(Yv, Yv, sv[:, None, :].to_broadcast([HOP, 2, 52]), MUL)
            nc.sync.dma_start(yv[:, :], Y[:, :])
    return nc


# ---------------- host side ----------------

def _host_consts():
    W, G = _dft_consts()
    fr_c = np.concatenate([W.real, W.imag], axis=1).astype(bfloat16)
    gr_c = np.concatenate([G.real, G.imag], axis=1).astype(bfloat16)
    cov = np.zeros(L)
    idx = (np.arange(T)[:, None] * HOP + np.arange(F)[None, :]).reshape(-1)
    np.add.at(cov, idx, 1.0)
    cov = np.where(cov > 0, cov, 1.0)
    return fr_c, gr_c, cov


def _smat_for(n2_list):
    S = np.zeros((NJ, F, F), np.float32)
    g = np.arange(F)
    for j, n2 in enumerate(n2_list):
        S[j, (g - n2) % F, g] = 1.0
    return np.ascontiguousarray(S.transpose(1, 0, 2).reshape(F, NJ * F)).astype(bfloat16)


def _mst_for(n2_list, w2):
    Ms = np.zeros((NJ, 2, F, F), np.float32)
    g = np.arange(F)[:, None]
    f = np.arange(F)[None, :]
    n1 = ((f - g + 20) % F) - 20
    valid = (n1 >= -20) & (n1 <= 19)
    n1c = np.clip(n1 + 20, 0, 39)
    for j, n2 in enumerate(n2_list):
        col = w2[:, n2 + 20]
        Ms[j, 0] = np.where(valid, col.real[n1c], 0.0)
        Ms[j, 1] = np.where(valid, col.imag[n1c], 0.0)
    return np.ascontiguousarray(
        Ms.transpose(2, 0, 1, 3).reshape(F, NJ * 2 * F)).astype(bfloat16)


def _frame(sig):
    idx = np.arange(T)[None, :] * HOP + np.arange(F)[:, None]   # [j, t]
    return sig[idx].astype(np.float32)


def make_in_maps(x_real, x_imag, task_info, w_real, w_imag):
    fr_c, gr_c, cov = _host_consts()
    b, _, m = x_real.shape
    P = np.power(10.0, task_info[:, 0] / 10.0) / m
    w2 = (np.asarray(w_real) + 1j * np.asarray(w_imag)).reshape(40, 40)
    smats = [_smat_for(nl) for nl in N2_LISTS]
    msts = [_mst_for(nl, w2) for nl in N2_LISTS]

    tp = np.arange(52)[None, :]
    tau = np.arange(HOP)[:, None]
    l = HOP * tp + tau
    svs = [(P[bb] / cov[l]).astype(np.float32) for bb in range(b)]

    in_maps, shards = [], []
    for bb in range(b):
        for mm in range(m):
            fr_ = _frame(x_real[bb, :, mm])
            fi_ = _frame(x_imag[bb, :, mm])
            xfv = np.concatenate([-fi_, fr_, fi_], axis=1).astype(bfloat16)
            for h in range(2):
                in_maps.append({
                    "xf": xfv,
                    "fr_c": fr_c,
                    "gr_c": gr_c,
                    "smat": smats[h],
                    "mst": msts[h],
                    "svec": svs[bb],
                })
                shards.append((bb, mm, h))
    return in_maps, shards, P, cov


_NC_CACHE = {}


def kernel(x_real, x_imag, task_info, w_real, w_imag, b_real, b_imag):
    x_real = np.asarray(x_real)
    x_imag = np.asarray(x_imag)
    task_info = np.asarray(task_info)
    b, Lx, m = x_real.shape
    assert (b, Lx, m) == (2, L, 2)

    if "nc" not in _NC_CACHE:
        nc_ = build_program(debug=False)
        nc_.compile()
        _NC_CACHE["nc"] = nc_
    nc = _NC_CACHE["nc"]

    in_maps, shards, P, cov = make_in_maps(x_real, x_imag, task_info, w_real, w_imag)
    from concourse.bass_utils import run_bass_kernel_spmd
    res = run_bass_kernel_spmd(nc, in_maps, list(range(8))).results

    x = (x_real + 1j * x_imag).astype(np.complex64)
    out = x.copy()
    bias = complex(np.asarray(b_real)[0], np.asarray(b_imag)[0])
    bias_sig = np.zeros(L, np.complex64)
    bias_sig[np.arange(T) * HOP] = bias
    bias_sig /= cov
    for i, (bb, mm, h) in enumerate(shards):
        yvv = res[i]["yv"]          # [40, 104] = [tau, (yr(52) | yi(52))]
        yr = yvv[:, 0:52].T.ravel()[:L]
        yi = yvv[:, 52:104].T.ravel()[:L]
        out[bb, :, mm] += yr + 1j * yi
    for bb in range(b):
        for mm in range(m):
            out[bb, :, mm] += (P[bb] * bias_sig).astype(np.complex64)
    return out[:, 20:L - 20, :]



# revision 7
# speedup vs baseline: 1.3390x; 1.3390x over previous
"""Trainium2 Bass kernel for nn_EqStftPBC (STFT perturbation-based compensation).

Per (batch b, mode m):
  X = STFT(x); C_n2 = X*conj(roll(X,n2)) + prev-frame; U_n2 = circulant(w[:,n2]);
  V_n2 = U_n2 * roll(X,n2); delta = sum_n2 G @ V_n2; out = x + ISTFT(delta)*P (+bias)

8 cores = (b x m x n2-half), uniform SPMD program; per-core variation only in
input data (permutation stack S, circulant stack M).  Device layout: [freq(80)
partitions, time free].  STFT fp32, rest bf16 (PSUM fp32).

v3: 4 chunks of 5 n2-planes, software-pipelined across engines.  All
TENSOR_TENSOR on Vector only (GpSimd shares Vector's 2nd SBUF port and fully
blocks it); Scalar does PSUM evictions + negations; V=U*R complex combine is
folded into +-G weight variants with PSUM accumulation; 5-plane D reduction is
one vector tensor_reduce over a stride-permuted PSUM view.
"""

import numpy as np
from ml_dtypes import bfloat16

import concourse.bass as bass
import concourse.bacc as bacc
import concourse.mybir as mybir
import concourse.tile as tile

F = 80
T = 51
TP = 52          # padded slot stride
HOP = 40
L = 2080
NJ = 20
NCH = 4
CHJ = NJ // NCH  # 5 planes per chunk
BL = CHJ * TP    # 260
FP32 = mybir.dt.float32
BF16 = mybir.dt.bfloat16

N2_LISTS = [list(range(19, -1, -1)), list(range(-1, -21, -1))]


def _dft_consts():
    j = np.arange(F)
    W = np.exp(-2j * np.pi * np.outer(j, j) / F)
    G = np.exp(+2j * np.pi * np.outer(j, j) / F) / F
    return W, G


def build_program(debug=False):
    nc = bacc.Bacc("TRN2", target_bir_lowering=False, debug=debug)

    # xf = [fiN | fr | fi] frames, pre-framed on host (pure reshape)
    xf = nc.dram_tensor("xf", [F, 3 * T], BF16, kind="ExternalInput")
    fr_c = nc.dram_tensor("fr_c", [F, 2 * F], BF16, kind="ExternalInput")
    gr_c = nc.dram_tensor("gr_c", [F, 4 * F], BF16, kind="ExternalInput")
    smat = nc.dram_tensor("smat", [F, NJ * F], BF16, kind="ExternalInput")
    mst = nc.dram_tensor("mst", [F, NJ * 2 * F], BF16, kind="ExternalInput")
    svec = nc.dram_tensor("svec", [HOP, 52], FP32, kind="ExternalInput")
    yv = nc.dram_tensor("yv", [HOP, 2 * 52], FP32, kind="ExternalOutput")

    with tile.TileContext(nc) as tc:
        with (
            tc.tile_pool(name="const", bufs=1) as cpool,
            tc.tile_pool(name="work", bufs=1) as wpool,
            tc.tile_pool(name="ps_x", bufs=1, space="PSUM") as ps_x,
            tc.tile_pool(name="ps_r", bufs=2, space="PSUM") as ps_r,
            tc.tile_pool(name="ps_u", bufs=2, space="PSUM") as ps_u,
            tc.tile_pool(name="ps_d", bufs=2, space="PSUM") as ps_d,
        ):
            TT = nc.vector.tensor_tensor
            MUL = mybir.AluOpType.mult
            ADD = mybir.AluOpType.add
            SUB = mybir.AluOpType.subtract
            CPY = mybir.ActivationFunctionType.Copy

            # ---- preamble: memset first, then few big DMAs per HWDGE queue
            Xsb = wpool.tile([F, 2 * TP], BF16, tag="Xsb")
            nc.gpsimd.memset(Xsb[:, :], 0)
            frm = wpool.tile([F, 3 * T], BF16, tag="frm")
            nc.sync.dma_start(frm[:, :], xf[:, :])
            Fc = cpool.tile([F, 2 * F], BF16, tag="Fc")
            nc.scalar.dma_start(Fc[:, :], fr_c[:, :])
            Ssb = cpool.tile([F, NJ * F], BF16, tag="Ssb")
            nc.sync.dma_start(Ssb[:, :], smat[:, :])
            Msb = cpool.tile([F, NJ * 2 * F], BF16, tag="Msb")
            nc.scalar.dma_start(Msb[:, 0:NJ * F], mst[:, 0:NJ * F])
            nc.gpsimd.dma_start(Msb[:, NJ * F:], mst[:, NJ * F:])
            Gc = cpool.tile([F, 4 * F], BF16, tag="Gc")
            nc.gpsimd.dma_start(Gc[:, :], gr_c[:, :])
            sv = cpool.tile([HOP, 52], FP32, tag="sv")
            nc.gpsimd.dma_start(sv[:, :], svec[:, :])

            # ---- STFT (fp32) -> X bf16 [Xr(52) | Xi(52)] ----
            Xp = ps_x.tile([F, 2 * T], FP32, tag="Xp")
            nc.tensor.matmul(Xp[:, :], Fc[:, 0:F], frm[:, T:3 * T], start=True, stop=False)
            nc.tensor.matmul(Xp[:, :], Fc[:, F:2 * F], frm[:, 0:2 * T], start=False, stop=True)
            Xsv = Xsb[:, :].rearrange("p (c t) -> p c t", c=2)
            nc.scalar.activation(Xsv[:, :, 0:T],
                                 Xp[:, :].rearrange("p (c t) -> p c t", c=2),
                                 CPY)
            Xrhs = bass.AP(tensor=Xsb[:, :].tensor, offset=Xsb[:, :].offset,
                           ap=[[2 * TP, F], [TP, 2], [1, T]])
            Xbr = Xsb[:, None, 0:TP].to_broadcast([F, CHJ, TP])
            Xbi = Xsb[:, None, TP:2 * TP].to_broadcast([F, CHJ, TP])

            # ---- per-chunk tiles ----
            Rsb = [wpool.tile([F, 2 * BL], BF16, tag=f"Rsb{c}", name=f"Rsb{c}")
                   for c in range(NCH)]
            Cc_ = [wpool.tile([F, 2 * BL], BF16, tag=f"Cc{c}", name=f"Cc{c}")
                   for c in range(NCH)]
            Cp_ = [wpool.tile([F, 3 * BL], BF16, tag=f"Cp{c}", name=f"Cp{c}")
                   for c in range(NCH)]
            Uc_ = [wpool.tile([F, 2 * BL], BF16, tag=f"Uc{c}", name=f"Uc{c}")
                   for c in range(NCH)]
            s4_ = [[wpool.tile([F, BL], BF16, tag=f"s{k}{c}", name=f"s{k}{c}")
                    for k in range(4)] for c in range(NCH)]
            v4_ = [[wpool.tile([F, BL], BF16, tag=f"v{k}{c}", name=f"v{k}{c}")
                    for k in range(4)] for c in range(NCH)]
            Dc_ = [wpool.tile([F, 2 * T], FP32, tag=f"Dc{c}", name=f"Dc{c}")
                   for c in range(NCH)]
            Dp_ = [None] * NCH

            # ---- emission helpers (each touches a single engine stream) ----
            def emit_R(c):          # PE: 5 perm matmuls -> ps_r bank
                Rp = ps_r.tile([F, CHJ * 2 * T], FP32, tag="Rp")
                for s in range(CHJ):
                    j = c * CHJ + s
                    nc.tensor.matmul(Rp[:, s * 2 * T:(s + 1) * 2 * T],
                                     Ssb[:, j * F:(j + 1) * F],
                                     Xrhs, start=True, stop=True)
                return Rp

            def emit_Rev(c, Rp):    # scalar: evict bank -> Rsb[c] plane-major
                dst = bass.AP(tensor=Rsb[c][:, :].tensor,
                              offset=Rsb[c][:, :].offset,
                              ap=[[2 * BL, F], [TP, CHJ], [BL, 2], [1, T]])
                nc.scalar.activation(
                    dst, Rp[:, :].rearrange("p (s c2 t) -> p s c2 t", s=CHJ, c2=2),
                    CPY)

            def emit_P(c):          # vector: 6 products + troll + wrap
                Rr = Rsb[c][:, 0:BL]
                Ri = Rsb[c][:, BL:2 * BL]
                Rr3 = Rr.rearrange("p (j t) -> p j t", j=CHJ)
                Ri3 = Ri.rearrange("p (j t) -> p j t", j=CHJ)
                sA, sB, sC, sD = s4_[c]
                Cc = Cc_[c]
                TT(sA[:, :].rearrange("p (j t) -> p j t", j=CHJ), Xbr, Rr3, MUL)
                TT(sB[:, :].rearrange("p (j t) -> p j t", j=CHJ), Xbi, Ri3, MUL)
                TT(Cc[:, 0:BL], sA[:, :], sB[:, :], ADD)
                TT(sC[:, :].rearrange("p (j t) -> p j t", j=CHJ), Xbi, Rr3, MUL)
                TT(sD[:, :].rearrange("p (j t) -> p j t", j=CHJ), Xbr, Ri3, MUL)
                TT(Cc[:, BL:2 * BL], sC[:, :], sD[:, :], SUB)
                # C' = C + roll_t(C) into Cp = [CiN' | Cr' | Ci']
                Cp = Cp_[c]
                d_in = bass.AP(tensor=Cc[:, :].tensor, offset=Cc[:, :].offset,
                               ap=[[2 * BL, F], [BL, 2], [TP, CHJ], [1, T - 1]])
                d_in1 = bass.AP(tensor=Cc[:, :].tensor, offset=Cc[:, :].offset + 1,
                                ap=[[2 * BL, F], [BL, 2], [TP, CHJ], [1, T - 1]])
                d_out = bass.AP(tensor=Cp[:, :].tensor, offset=Cp[:, :].offset + BL + 1,
                                ap=[[3 * BL, F], [BL, 2], [TP, CHJ], [1, T - 1]])
                TT(d_out, d_in1, d_in, ADD)
                w_in = bass.AP(tensor=Cc[:, :].tensor, offset=Cc[:, :].offset,
                               ap=[[2 * BL, F], [BL, 2], [TP, CHJ], [1, 1]])
                w_in1 = bass.AP(tensor=Cc[:, :].tensor, offset=Cc[:, :].offset + T - 1,
                                ap=[[2 * BL, F], [BL, 2], [TP, CHJ], [1, 1]])
                w_out = bass.AP(tensor=Cp[:, :].tensor, offset=Cp[:, :].offset + BL,
                                ap=[[3 * BL, F], [BL, 2], [TP, CHJ], [1, 1]])
                TT(w_out, w_in, w_in1, ADD)

            def emit_CiN(c):        # scalar: CiN' = -Ci'
                Cp = Cp_[c]
                nc.scalar.activation(Cp[:, 0:BL], Cp[:, 2 * BL:3 * BL], CPY, scale=-1.0)

            def emit_S1(c):         # PE: 10 matmuls -> ps_u bank
                Cp = Cp_[c]
                Up = ps_u.tile([F, CHJ * 2 * T], FP32, tag="Up")
                for s in range(CHJ):
                    j = c * CHJ + s
                    rhs1 = bass.AP(tensor=Cp[:, :].tensor,
                                   offset=Cp[:, :].offset + BL + s * TP,
                                   ap=[[3 * BL, F], [BL, 2], [1, T]])
                    rhs2 = bass.AP(tensor=Cp[:, :].tensor,
                                   offset=Cp[:, :].offset + s * TP,
                                   ap=[[3 * BL, F], [BL, 2], [1, T]])
                    nc.tensor.matmul(Up[:, s * 2 * T:(s + 1) * 2 * T],
                                     Msb[:, (2 * j) * F:(2 * j + 1) * F],
                                     rhs1, start=True, stop=False)
                    nc.tensor.matmul(Up[:, s * 2 * T:(s + 1) * 2 * T],
                                     Msb[:, (2 * j + 1) * F:(2 * j + 2) * F],
                                     rhs2, start=False, stop=True)
                return Up

            def emit_Uev(c, Up):    # scalar: evict bank -> Uc[c]
                dst = bass.AP(tensor=Uc_[c][:, :].tensor,
                              offset=Uc_[c][:, :].offset,
                              ap=[[2 * BL, F], [TP, CHJ], [BL, 2], [1, T]])
                nc.scalar.activation(
                    dst, Up[:, :].rearrange("p (s c2 t) -> p s c2 t", s=CHJ, c2=2),
                    CPY)

            def emit_V(c):          # vector: 4 products
                Rr = Rsb[c][:, 0:BL]
                Ri = Rsb[c][:, BL:2 * BL]
                Ur = Uc_[c][:, 0:BL]
                Ui = Uc_[c][:, BL:2 * BL]
                vA, vB, vC, vD = v4_[c]
                TT(vA[:, :], Ur, Rr, MUL)
                TT(vB[:, :], Ui, Ri, MUL)
                TT(vC[:, :], Ur, Ri, MUL)
                TT(vD[:, :], Ui, Rr, MUL)

            def emit_G(c):          # PE: 8 matmuls accumulate -> ps_d bank
                vA, vB, vC, vD = v4_[c]
                Dp = ps_d.tile([F, CHJ * 2 * T], FP32, tag="Dp")
                Dp_[c] = Dp
                Gr = Gc[:, 0:F]
                Gi = Gc[:, F:2 * F]
                Grn = Gc[:, 2 * F:3 * F]
                Gin = Gc[:, 3 * F:4 * F]

                def vrhs(tile_):
                    return bass.AP(tensor=tile_[:, :].tensor,
                                   offset=tile_[:, :].offset,
                                   ap=[[BL, F], [TP, CHJ], [1, T]])

                def dout(c2):
                    return bass.AP(tensor=Dp[:, :].tensor,
                                   offset=Dp[:, :].offset + c2 * T,
                                   ap=[[CHJ * 2 * T, F], [2 * T, CHJ], [1, T]])

                calls = [
                    (0, Gr, vA), (0, Grn, vB), (0, Gin, vC), (0, Gin, vD),
                    (1, Gr, vC), (1, Gr, vD), (1, Gi, vA), (1, Gin, vB),
                ]
                # start=True clears the whole bank's has_written: exactly one
                # start (first call) and one stop (last call)
                for i, (c2, w, v) in enumerate(calls):
                    nc.tensor.matmul(dout(c2), w, vrhs(v),
                                     start=(i == 0), stop=(i == len(calls) - 1))

            def emit_red(c):        # vector: 5-plane reduce -> Dc[c]
                Dp = Dp_[c]
                rin = bass.AP(tensor=Dp[:, :].tensor, offset=Dp[:, :].offset,
                              ap=[[CHJ * 2 * T, F], [1, 2 * T], [2 * T, CHJ]])
                nc.vector.tensor_reduce(
                    Dc_[c][:, :].rearrange("p (ct one) -> p ct one", one=1),
                    rin, axis=mybir.AxisListType.X, op=ADD)

            # ---- pipelined emission ----
            Rps = []
            for c in range(NCH):
                Rps.append(emit_R(c))
            emit_Rev(0, Rps[0])
            emit_Rev(1, Rps[1])
            emit_P(0)
            emit_CiN(0)
            Up0 = emit_S1(0)
            emit_Rev(2, Rps[2])
            emit_P(1)
            emit_CiN(1)
            emit_Uev(0, Up0)
            Up1 = emit_S1(1)
            emit_Rev(3, Rps[3])
            emit_P(2)
            emit_V(0)
            emit_G(0)
            emit_CiN(2)
            emit_Uev(1, Up1)
            Up2 = emit_S1(2)
            emit_P(3)
            emit_V(1)
            emit_G(1)
            emit_red(0)
            emit_CiN(3)
            emit_Uev(2, Up2)
            Up3 = emit_S1(3)
            emit_V(2)
            emit_G(2)
            emit_red(1)
            emit_Uev(3, Up3)
            emit_V(3)
            emit_G(3)
            D01 = wpool.tile([F, 2 * T], FP32, tag="D01")
            TT(D01[:, :], Dc_[0][:, :], Dc_[1][:, :], ADD)
            emit_red(2)
            emit_red(3)
            D012 = wpool.tile([F, 2 * T], FP32, tag="D012")
            TT(D012[:, :], D01[:, :], Dc_[2][:, :], ADD)
            Dsb = wpool.tile([F, 2 * T], FP32, tag="Dsb")
            TT(Dsb[:, :], D012[:, :], Dc_[3][:, :], ADD)

            # ---- overlap-add + scale (fp32) ----
            S2 = wpool.tile([HOP, 2 * T], FP32, tag="S2")
            nc.sync.dma_start(S2[:, :], Dsb[HOP:F, :])
            Y = wpool.tile([HOP, 2 * 52], FP32, tag="Y")
            S1v = Dsb[0:HOP, :].rearrange("p (c t) -> p c t", c=2)
            S2v = S2[:, :].rearrange("p (c t) -> p c t", c=2)
            Yv = Y[:, :].rearrange("p (c t) -> p c t", c=2)
            TT(Yv[:, :, 1:T], S1v[:, :, 1:T], S2v[:, :, 0:T - 1], ADD)
            nc.scalar.activation(Yv[:, :, 0:1], S1v[:, :, 0:1], CPY)
            nc.scalar.activation(Yv[:, :, T:52], S2v[:, :, T - 1:T], CPY)
            TT(Yv, Yv, sv[:, None, :].to_broadcast([HOP, 2, 52]), MUL)
            nc.sync.dma_start(yv[:, :], Y[:, :])
    return nc


# ---------------- host side ----------------

def _host_consts():
    W, G = _dft_consts()
    fr_c = np.concatenate([W.real, W.imag], axis=1).astype(bfloat16)
    gr_c = np.concatenate([G.real, G.imag, -G.real, -G.imag], axis=1).astype(bfloat16)
    cov = np.zeros(L)
    idx = (np.arange(T)[:, None] * HOP + np.arange(F)[None, :]).reshape(-1)
    np.add.at(cov, idx, 1.0)
    cov = np.where(cov > 0, cov, 1.0)
    return fr_c, gr_c, cov


def _smat_for(n2_list):
    S = np.zeros((NJ, F, F), np.float32)
    g = np.arange(F)
    for j, n2 in enumerate(n2_list):
        S[j, (g - n2) % F, g] = 1.0
    return np.ascontiguousarray(S.transpose(1, 0, 2).reshape(F, NJ * F)).astype(bfloat16)


def _mst_for(n2_list, w2):
    Ms = np.zeros((NJ, 2, F, F), np.float32)
    g = np.arange(F)[:, None]
    f = np.arange(F)[None, :]
    n1 = ((f - g + 20) % F) - 20
    valid = (n1 >= -20) & (n1 <= 19)
    n1c = np.clip(n1 + 20, 0, 39)
    for j, n2 in enumerate(n2_list):
        col = w2[:, n2 + 20]
        Ms[j, 0] = np.where(valid, col.real[n1c], 0.0)
        Ms[j, 1] = np.where(valid, col.imag[n1c], 0.0)
    return np.ascontiguousarray(
        Ms.transpose(2, 0, 1, 3).reshape(F, NJ * 2 * F)).astype(bfloat16)


def _frame(sig):
    idx = np.arange(T)[None, :] * HOP + np.arange(F)[:, None]   # [j, t]
    return sig[idx].astype(np.float32)


def make_in_maps(x_real, x_imag, task_info, w_real, w_imag):
    fr_c, gr_c, cov = _host_consts()
    b, _, m = x_real.shape
    P = np.power(10.0, task_info[:, 0] / 10.0) / m
    w2 = (np.asarray(w_real) + 1j * np.asarray(w_imag)).reshape(40, 40)
    smats = [_smat_for(nl) for nl in N2_LISTS]
    msts = [_mst_for(nl, w2) for nl in N2_LISTS]

    tp = np.arange(52)[None, :]
    tau = np.arange(HOP)[:, None]
    l = HOP * tp + tau
    svs = [(P[bb] / cov[l]).astype(np.float32) for bb in range(b)]

    in_maps, shards = [], []
    for bb in range(b):
        for mm in range(m):
            fr_ = _frame(x_real[bb, :, mm])
            fi_ = _frame(x_imag[bb, :, mm])
            xfv = np.concatenate([-fi_, fr_, fi_], axis=1).astype(bfloat16)
            for h in range(2):
                in_maps.append({
                    "xf": xfv,
                    "fr_c": fr_c,
                    "gr_c": gr_c,
                    "smat": smats[h],
                    "mst": msts[h],
                    "svec": svs[bb],
                })
                shards.append((bb, mm, h))
    return in_maps, shards, P, cov


_NC_CACHE = {}


def kernel(x_real, x_imag, task_info, w_real, w_imag, b_real, b_imag):
    x_real = np.asarray(x_real)
    x_imag = np.asarray(x_imag)
    task_info = np.asarray(task_info)
    b, Lx, m = x_real.shape
    assert (b, Lx, m) == (2, L, 2)

    if "nc" not in _NC_CACHE:
        nc_ = build_program(debug=False)
        nc_.compile()
        _NC_CACHE["nc"] = nc_
    nc = _NC_CACHE["nc"]

    in_maps, shards, P, cov = make_in_maps(x_real, x_imag, task_info, w_real, w_imag)
    from concourse.bass_utils import run_bass_kernel_spmd
    res = run_bass_kernel_spmd(nc, in_maps, list(range(8))).results

    x = (x_real + 1j * x_imag).astype(np.complex64)
    out = x.copy()
    bias = complex(np.asarray(b_real)[0], np.asarray(b_imag)[0])
    bias_sig = np.zeros(L, np.complex64)
    bias_sig[np.arange(T) * HOP] = bias
    bias_sig /= cov
    for i, (bb, mm, h) in enumerate(shards):
        yvv = res[i]["yv"]          # [40, 104] = [tau, (yr(52) | yi(52))]
        yr = yvv[:, 0:52].T.ravel()[:L]
        yi = yvv[:, 52:104].T.ravel()[:L]
        out[bb, :, mm] += yr + 1j * yi
    for bb in range(b):
        for mm in range(m):
            out[bb, :, mm] += (P[bb] * bias_sig).astype(np.complex64)
    return out[:, 20:L - 20, :]


# revision 16
# speedup vs baseline: 1.4263x; 1.0653x over previous
"""Trainium2 Bass kernel for nn_EqStftPBC (STFT perturbation-based compensation).

Per (batch b, mode m):
  X = STFT(x); C_n2 = X*conj(roll(X,n2)) + prev-frame; U_n2 = circulant(w[:,n2]);
  V_n2 = U_n2 * roll(X,n2); delta = sum_n2 G @ V_n2; out = x + ISTFT(delta)*P (+bias)

8 cores = (b x m x n2-half), uniform SPMD program; per-core variation only in
input data (permutation stack S, circulant stack M).  Device layout: [freq(80)
partitions, time free].  STFT fp32, rest bf16 (PSUM fp32).

v3: 4 chunks of 5 n2-planes, software-pipelined across engines.  All
TENSOR_TENSOR on Vector only (GpSimd shares Vector's 2nd SBUF port and fully
blocks it); Scalar does PSUM evictions + negations; V=U*R complex combine is
folded into +-G weight variants with PSUM accumulation; 5-plane D reduction is
one vector tensor_reduce over a stride-permuted PSUM view.
"""

import numpy as np
from ml_dtypes import bfloat16

import concourse.bass as bass
import concourse.bacc as bacc
import concourse.mybir as mybir
import concourse.tile as tile

F = 80
T = 51
TP = 52          # padded slot stride
HOP = 40
L = 2080
NJ = 20
NCH = 4
CHJ = NJ // NCH  # 5 planes per chunk
BL = CHJ * TP    # 260
FP32 = mybir.dt.float32
BF16 = mybir.dt.bfloat16

N2_LISTS = [list(range(19, -1, -1)), list(range(-1, -21, -1))]


def _dft_consts():
    j = np.arange(F)
    W = np.exp(-2j * np.pi * np.outer(j, j) / F)
    G = np.exp(+2j * np.pi * np.outer(j, j) / F) / F
    return W, G


def build_program(debug=False):
    nc = bacc.Bacc("TRN2", target_bir_lowering=False, debug=debug)

    # xf = [fiN | fr | fi] frames, pre-framed on host (pure reshape)
    xf = nc.dram_tensor("xf", [F, 3 * T], BF16, kind="ExternalInput")
    fr_c = nc.dram_tensor("fr_c", [F, 2 * F], BF16, kind="ExternalInput")
    gr_c = nc.dram_tensor("gr_c", [F, 4 * F], BF16, kind="ExternalInput")
    smat = nc.dram_tensor("smat", [F, NJ * F], BF16, kind="ExternalInput")
    mst = nc.dram_tensor("mst", [F, NJ * 2 * F], BF16, kind="ExternalInput")
    svec = nc.dram_tensor("svec", [HOP, 52], FP32, kind="ExternalInput")
    psh = nc.dram_tensor("psh", [F, HOP], FP32, kind="ExternalInput")
    yv = nc.dram_tensor("yv", [HOP, 2 * 52], FP32, kind="ExternalOutput")

    with tile.TileContext(nc) as tc:
        with (
            tc.tile_pool(name="const", bufs=1) as cpool,
            tc.tile_pool(name="work", bufs=1) as wpool,
            tc.tile_pool(name="ps_x", bufs=1, space="PSUM") as ps_x,
            tc.tile_pool(name="ps_r", bufs=2, space="PSUM") as ps_r,
            tc.tile_pool(name="ps_u", bufs=2, space="PSUM") as ps_u,
            tc.tile_pool(name="ps_d", bufs=1, space="PSUM") as ps_d,
        ):
            TT = nc.vector.tensor_tensor
            MUL = mybir.AluOpType.mult
            ADD = mybir.AluOpType.add
            SUB = mybir.AluOpType.subtract
            CPY = mybir.ActivationFunctionType.Copy

            # ---- preamble: DMAs chunk-split across the 3 HWDGE/SWDGE rings so
            # the first consumers unblock earliest (smat chunk c feeds R(c))
            Xsb = wpool.tile([F, 2 * TP], BF16, tag="Xsb")
            nc.gpsimd.memset(Xsb[:, :], 0)
            frm = wpool.tile([F, 3 * T], BF16, tag="frm")
            nc.sync.dma_start(frm[:, :], xf[:, :])
            Fc = cpool.tile([F, 2 * F], BF16, tag="Fc")
            nc.scalar.dma_start(Fc[:, :], fr_c[:, :])
            Ssb = cpool.tile([F, NJ * F], BF16, tag="Ssb")
            Q = NJ * F // 4
            nc.sync.dma_start(Ssb[:, 0:2 * Q], smat[:, 0:2 * Q])
            nc.scalar.dma_start(Ssb[:, 2 * Q:3 * Q], smat[:, 2 * Q:3 * Q])
            nc.gpsimd.dma_start(Ssb[:, 3 * Q:4 * Q], smat[:, 3 * Q:4 * Q])
            Msb = cpool.tile([F, NJ * 2 * F], BF16, tag="Msb")
            M4 = NJ * 2 * F // 4
            nc.scalar.dma_start(Msb[:, 0:M4], mst[:, 0:M4])
            nc.gpsimd.dma_start(Msb[:, M4:2 * M4], mst[:, M4:2 * M4])
            nc.sync.dma_start(Msb[:, 2 * M4:], mst[:, 2 * M4:])
            Gc = cpool.tile([F, 4 * F], BF16, tag="Gc")
            nc.gpsimd.dma_start(Gc[:, :], gr_c[:, :])
            sv = cpool.tile([HOP, 52], FP32, tag="sv")
            nc.gpsimd.dma_start(sv[:, :], svec[:, :])
            Psh = cpool.tile([F, HOP], FP32, tag="Psh")
            nc.gpsimd.dma_start(Psh[:, :], psh[:, :])

            # ---- STFT (fp32) -> X bf16 [Xr(52) | Xi(52)] ----
            Xp = ps_x.tile([F, 2 * T], FP32, tag="Xp")
            nc.tensor.matmul(Xp[:, :], Fc[:, 0:F], frm[:, T:3 * T], start=True, stop=False)
            nc.tensor.matmul(Xp[:, :], Fc[:, F:2 * F], frm[:, 0:2 * T], start=False, stop=True)
            Xsv = Xsb[:, :].rearrange("p (c t) -> p c t", c=2)
            nc.scalar.activation(Xsv[:, :, 0:T],
                                 Xp[:, :].rearrange("p (c t) -> p c t", c=2),
                                 CPY)
            Xrhs = bass.AP(tensor=Xsb[:, :].tensor, offset=Xsb[:, :].offset,
                           ap=[[2 * TP, F], [TP, 2], [1, T]])
            Xbr = Xsb[:, None, 0:TP].to_broadcast([F, CHJ, TP])
            Xbi = Xsb[:, None, TP:2 * TP].to_broadcast([F, CHJ, TP])

            # ---- per-chunk tiles ----
            Rsb = [wpool.tile([F, 2 * BL], BF16, tag=f"Rsb{c}", name=f"Rsb{c}")
                   for c in range(NCH)]
            Cc_ = [wpool.tile([F, 2 * BL], BF16, tag=f"Cc{c}", name=f"Cc{c}")
                   for c in range(NCH)]
            Cp_ = [wpool.tile([F, 3 * BL], BF16, tag=f"Cp{c}", name=f"Cp{c}")
                   for c in range(NCH)]
            Uc_ = [wpool.tile([F, 2 * BL], BF16, tag=f"Uc{c}", name=f"Uc{c}")
                   for c in range(NCH)]
            s4_ = [[wpool.tile([F, BL], BF16, tag=f"s{k}{c}", name=f"s{k}{c}")
                    for k in range(4)] for c in range(NCH)]
            v4_ = [[wpool.tile([F, BL], BF16, tag=f"v{k}{c}", name=f"v{k}{c}")
                    for k in range(4)] for c in range(NCH)]
            Dc_ = [wpool.tile([F, 2 * T], FP32, tag=f"Dc{p}", name=f"Dc{p}")
                   for p in range(2)]
            Dp_ = [None] * 2

            # ---- emission helpers (each touches a single engine stream) ----
            def emit_R(c):          # PE: 5 perm matmuls -> ps_r bank
                Rp = ps_r.tile([F, CHJ * 2 * T], FP32, tag="Rp")
                for s in range(CHJ):
                    j = c * CHJ + s
                    nc.tensor.matmul(Rp[:, s * 2 * T:(s + 1) * 2 * T],
                                     Ssb[:, j * F:(j + 1) * F],
                                     Xrhs, start=True, stop=True)
                return Rp

            def emit_Rev(c, Rp):    # scalar: evict bank -> Rsb[c] plane-major
                dst = bass.AP(tensor=Rsb[c][:, :].tensor,
                              offset=Rsb[c][:, :].offset,
                              ap=[[2 * BL, F], [TP, CHJ], [BL, 2], [1, T]])
                nc.scalar.activation(
                    dst, Rp[:, :].rearrange("p (s c2 t) -> p s c2 t", s=CHJ, c2=2),
                    CPY)

            def emit_P(c):          # vector: 6 products + troll + wrap
                Rr = Rsb[c][:, 0:BL]
                Ri = Rsb[c][:, BL:2 * BL]
                Rr3 = Rr.rearrange("p (j t) -> p j t", j=CHJ)
                Ri3 = Ri.rearrange("p (j t) -> p j t", j=CHJ)
                sA, sB, sC, sD = s4_[c]
                Cc = Cc_[c]
                TT(sA[:, :].rearrange("p (j t) -> p j t", j=CHJ), Xbr, Rr3, MUL)
                TT(sB[:, :].rearrange("p (j t) -> p j t", j=CHJ), Xbi, Ri3, MUL)
                TT(Cc[:, 0:BL], sA[:, :], sB[:, :], ADD)
                TT(sC[:, :].rearrange("p (j t) -> p j t", j=CHJ), Xbi, Rr3, MUL)
                TT(sD[:, :].rearrange("p (j t) -> p j t", j=CHJ), Xbr, Ri3, MUL)
                TT(Cc[:, BL:2 * BL], sC[:, :], sD[:, :], SUB)
                # C' = C + roll_t(C) into Cp = [CiN' | Cr' | Ci']
                Cp = Cp_[c]
                d_in = bass.AP(tensor=Cc[:, :].tensor, offset=Cc[:, :].offset,
                               ap=[[2 * BL, F], [BL, 2], [TP, CHJ], [1, T - 1]])
                d_in1 = bass.AP(tensor=Cc[:, :].tensor, offset=Cc[:, :].offset + 1,
                                ap=[[2 * BL, F], [BL, 2], [TP, CHJ], [1, T - 1]])
                d_out = bass.AP(tensor=Cp[:, :].tensor, offset=Cp[:, :].offset + BL + 1,
                                ap=[[3 * BL, F], [BL, 2], [TP, CHJ], [1, T - 1]])
                TT(d_out, d_in1, d_in, ADD)
                w_in = bass.AP(tensor=Cc[:, :].tensor, offset=Cc[:, :].offset,
                               ap=[[2 * BL, F], [BL, 2], [TP, CHJ], [1, 1]])
                w_in1 = bass.AP(tensor=Cc[:, :].tensor, offset=Cc[:, :].offset + T - 1,
                                ap=[[2 * BL, F], [BL, 2], [TP, CHJ], [1, 1]])
                w_out = bass.AP(tensor=Cp[:, :].tensor, offset=Cp[:, :].offset + BL,
                                ap=[[3 * BL, F], [BL, 2], [TP, CHJ], [1, 1]])
                TT(w_out, w_in, w_in1, ADD)

            def emit_CiN(c):        # scalar: CiN' = -Ci'
                Cp = Cp_[c]
                nc.scalar.activation(Cp[:, 0:BL], Cp[:, 2 * BL:3 * BL], CPY, scale=-1.0)

            def emit_S1(c):         # PE: 10 matmuls -> ps_u bank
                Cp = Cp_[c]
                Up = ps_u.tile([F, CHJ * 2 * T], FP32, tag="Up")
                for s in range(CHJ):
                    j = c * CHJ + s
                    rhs1 = bass.AP(tensor=Cp[:, :].tensor,
                                   offset=Cp[:, :].offset + BL + s * TP,
                                   ap=[[3 * BL, F], [BL, 2], [1, T]])
                    rhs2 = bass.AP(tensor=Cp[:, :].tensor,
                                   offset=Cp[:, :].offset + s * TP,
                                   ap=[[3 * BL, F], [BL, 2], [1, T]])
                    nc.tensor.matmul(Up[:, s * 2 * T:(s + 1) * 2 * T],
                                     Msb[:, (2 * j) * F:(2 * j + 1) * F],
                                     rhs1, start=True, stop=False)
                    nc.tensor.matmul(Up[:, s * 2 * T:(s + 1) * 2 * T],
                                     Msb[:, (2 * j + 1) * F:(2 * j + 2) * F],
                                     rhs2, start=False, stop=True)
                return Up

            def emit_Uev(c, Up):    # scalar: evict bank -> Uc[c]
                dst = bass.AP(tensor=Uc_[c][:, :].tensor,
                              offset=Uc_[c][:, :].offset,
                              ap=[[2 * BL, F], [TP, CHJ], [BL, 2], [1, T]])
                nc.scalar.activation(
                    dst, Up[:, :].rearrange("p (s c2 t) -> p s c2 t", s=CHJ, c2=2),
                    CPY)

            def emit_V(c):          # vector: 4 products
                Rr = Rsb[c][:, 0:BL]
                Ri = Rsb[c][:, BL:2 * BL]
                Ur = Uc_[c][:, 0:BL]
                Ui = Uc_[c][:, BL:2 * BL]
                vA, vB, vC, vD = v4_[c]
                TT(vA[:, :], Ur, Rr, MUL)
                TT(vB[:, :], Ui, Ri, MUL)
                TT(vC[:, :], Ur, Ri, MUL)
                TT(vD[:, :], Ui, Rr, MUL)

            def emit_G(c):          # PE: 8 matmuls; chunk pair (2p, 2p+1)
                # accumulates into one shared ps_d bank
                vA, vB, vC, vD = v4_[c]
                p = c // 2
                if c % 2 == 0:
                    Dp_[p] = ps_d.tile([F, CHJ * 2 * T], FP32, tag=f"Dp{p}",
                                       name=f"Dp{p}")
                Dp = Dp_[p]
                Gr = Gc[:, 0:F]
                Gi = Gc[:, F:2 * F]
                Grn = Gc[:, 2 * F:3 * F]
                Gin = Gc[:, 3 * F:4 * F]

                def vrhs(tile_):
                    return bass.AP(tensor=tile_[:, :].tensor,
                                   offset=tile_[:, :].offset,
                                   ap=[[BL, F], [TP, CHJ], [1, T]])

                def dout(c2):
                    return bass.AP(tensor=Dp[:, :].tensor,
                                   offset=Dp[:, :].offset + c2 * T,
                                   ap=[[CHJ * 2 * T, F], [2 * T, CHJ], [1, T]])

                calls = [
                    (0, Gr, vA), (0, Grn, vB), (0, Gin, vC), (0, Gin, vD),
                    (1, Gr, vC), (1, Gr, vD), (1, Gi, vA), (1, Gin, vB),
                ]
                # one start (first call of even chunk) and one stop (last call
                # of odd chunk) per bank: start clears the whole bank
                for i, (c2, w, v) in enumerate(calls):
                    nc.tensor.matmul(dout(c2), w, vrhs(v),
                                     start=(c % 2 == 0 and i == 0),
                                     stop=(c % 2 == 1 and i == len(calls) - 1))

            def emit_red(p):        # vector: 5-plane reduce -> Dc[p]
                Dp = Dp_[p]
                rin = bass.AP(tensor=Dp[:, :].tensor, offset=Dp[:, :].offset,
                              ap=[[CHJ * 2 * T, F], [1, 2 * T], [2 * T, CHJ]])
                nc.vector.tensor_reduce(
                    Dc_[p][:, :].rearrange("p (ct one) -> p ct one", one=1),
                    rin, axis=mybir.AxisListType.X, op=ADD)

            def emit_shift(p, Sp):  # PE: Sp += Dc[p][40:80, :] to partitions 0:40
                nc.tensor.matmul(Sp[:, :], Psh[:, :], Dc_[p][:, :],
                                 start=(p == 0), stop=(p == 1))

            # ---- pipelined emission ----
            Rps = []
            for c in range(NCH):
                Rps.append(emit_R(c))
            emit_Rev(0, Rps[0])
            emit_Rev(1, Rps[1])
            emit_P(0)
            emit_CiN(0)
            Up0 = emit_S1(0)
            emit_Rev(2, Rps[2])
            emit_P(1)
            emit_CiN(1)
            emit_Uev(0, Up0)
            Up1 = emit_S1(1)
            emit_Rev(3, Rps[3])
            emit_P(2)
            emit_V(0)
            emit_G(0)
            emit_CiN(2)
            emit_Uev(1, Up1)
            Up2 = emit_S1(2)
            emit_P(3)
            emit_V(1)
            emit_G(1)
            emit_CiN(3)
            emit_Uev(2, Up2)
            Up3 = emit_S1(3)
            emit_V(2)
            emit_G(2)
            emit_red(0)
            emit_Uev(3, Up3)
            emit_V(3)
            Sp = ps_x.tile([HOP, 2 * T], FP32, tag="Sp")
            emit_G(3)
            emit_shift(0, Sp)
            emit_red(1)
            emit_shift(1, Sp)

            # ---- overlap-add + scale (fp32); bottom half arrives via the
            # PE partition-shift matmul accumulated in PSUM (Sp)
            Dtop = wpool.tile([HOP, 2 * T], FP32, tag="Dtop")
            TT(Dtop[:, :], Dc_[0][0:HOP, :], Dc_[1][0:HOP, :], ADD)
            Y = wpool.tile([HOP, 2 * 52], FP32, tag="Y")
            S1v = Dtop[:, :].rearrange("p (c t) -> p c t", c=2)
            S2v = Sp[:, :].rearrange("p (c t) -> p c t", c=2)
            Yv = Y[:, :].rearrange("p (c t) -> p c t", c=2)
            TT(Yv[:, :, 1:T], S1v[:, :, 1:T], S2v[:, :, 0:T - 1], ADD)
            nc.scalar.activation(Yv[:, :, 0:1], S1v[:, :, 0:1], CPY)
            nc.scalar.activation(Yv[:, :, T:52], S2v[:, :, T - 1:T], CPY)
            TT(Y[:, 0:52], Y[:, 0:52], sv[:, :], MUL)
            nc.sync.dma_start(yv[:, 0:52], Y[:, 0:52])
            TT(Y[:, 52:104], Y[:, 52:104], sv[:, :], MUL)
            nc.scalar.dma_start(yv[:, 52:104], Y[:, 52:104])
    return nc


# ---------------- host side ----------------

def _host_consts():
    W, G = _dft_consts()
    fr_c = np.concatenate([W.real, W.imag], axis=1).astype(bfloat16)
    gr_c = np.concatenate([G.real, G.imag, -G.real, -G.imag], axis=1).astype(bfloat16)
    psh = np.zeros((F, HOP), np.float32)
    psh[np.arange(HOP) + HOP, np.arange(HOP)] = 1.0
    cov = np.zeros(L)
    idx = (np.arange(T)[:, None] * HOP + np.arange(F)[None, :]).reshape(-1)
    np.add.at(cov, idx, 1.0)
    cov = np.where(cov > 0, cov, 1.0)
    return fr_c, gr_c, psh, cov


def _smat_for(n2_list):
    S = np.zeros((NJ, F, F), np.float32)
    g = np.arange(F)
    for j, n2 in enumerate(n2_list):
        S[j, (g - n2) % F, g] = 1.0
    return np.ascontiguousarray(S.transpose(1, 0, 2).reshape(F, NJ * F)).astype(bfloat16)


def _mst_for(n2_list, w2):
    Ms = np.zeros((NJ, 2, F, F), np.float32)
    g = np.arange(F)[:, None]
    f = np.arange(F)[None, :]
    n1 = ((f - g + 20) % F) - 20
    valid = (n1 >= -20) & (n1 <= 19)
    n1c = np.clip(n1 + 20, 0, 39)
    for j, n2 in enumerate(n2_list):
        col = w2[:, n2 + 20]
        Ms[j, 0] = np.where(valid, col.real[n1c], 0.0)
        Ms[j, 1] = np.where(valid, col.imag[n1c], 0.0)
    return np.ascontiguousarray(
        Ms.transpose(2, 0, 1, 3).reshape(F, NJ * 2 * F)).astype(bfloat16)


def _frame(sig):
    idx = np.arange(T)[None, :] * HOP + np.arange(F)[:, None]   # [j, t]
    return sig[idx].astype(np.float32)


def make_in_maps(x_real, x_imag, task_info, w_real, w_imag):
    fr_c, gr_c, psh, cov = _host_consts()
    b, _, m = x_real.shape
    P = np.power(10.0, task_info[:, 0] / 10.0) / m
    w2 = (np.asarray(w_real) + 1j * np.asarray(w_imag)).reshape(40, 40)
    smats = [_smat_for(nl) for nl in N2_LISTS]
    msts = [_mst_for(nl, w2) for nl in N2_LISTS]

    tp = np.arange(52)[None, :]
    tau = np.arange(HOP)[:, None]
    l = HOP * tp + tau
    svs = [(P[bb] / cov[l]).astype(np.float32) for bb in range(b)]

    in_maps, shards = [], []
    for bb in range(b):
        for mm in range(m):
            fr_ = _frame(x_real[bb, :, mm])
            fi_ = _frame(x_imag[bb, :, mm])
            xfv = np.concatenate([-fi_, fr_, fi_], axis=1).astype(bfloat16)
            for h in range(2):
                in_maps.append({
                    "xf": xfv,
                    "fr_c": fr_c,
                    "gr_c": gr_c,
                    "smat": smats[h],
                    "mst": msts[h],
                    "svec": svs[bb],
                    "psh": psh,
                })
                shards.append((bb, mm, h))
    return in_maps, shards, P, cov


_NC_CACHE = {}


def kernel(x_real, x_imag, task_info, w_real, w_imag, b_real, b_imag):
    x_real = np.asarray(x_real)
    x_imag = np.asarray(x_imag)
    task_info = np.asarray(task_info)
    b, Lx, m = x_real.shape
    assert (b, Lx, m) == (2, L, 2)

    if "nc" not in _NC_CACHE:
        nc_ = build_program(debug=False)
        nc_.compile()
        _NC_CACHE["nc"] = nc_
    nc = _NC_CACHE["nc"]

    in_maps, shards, P, cov = make_in_maps(x_real, x_imag, task_info, w_real, w_imag)
    from concourse.bass_utils import run_bass_kernel_spmd
    res = run_bass_kernel_spmd(nc, in_maps, list(range(8))).results

    x = (x_real + 1j * x_imag).astype(np.complex64)
    out = x.copy()
    bias = complex(np.asarray(b_real)[0], np.asarray(b_imag)[0])
    bias_sig = np.zeros(L, np.complex64)
    bias_sig[np.arange(T) * HOP] = bias
    bias_sig /= cov
    for i, (bb, mm, h) in enumerate(shards):
        yvv = res[i]["yv"]          # [40, 104] = [tau, (yr(52) | yi(52))]
        yr = yvv[:, 0:52].T.ravel()[:L]
        yi = yvv[:, 52:104].T.ravel()[:L]
        out[bb, :, mm] += yr + 1j * yi
    for bb in range(b):
        for mm in range(m):
            out[bb, :, mm] += (P[bb] * bias_sig).astype(np.complex64)
    return out[:, 20:L - 20, :]


# revision 21
# speedup vs baseline: 1.5631x; 1.0959x over previous
"""Trainium2 Bass kernel for nn_EqStftPBC (STFT perturbation-based compensation).

Per (batch b, mode m):
  X = STFT(x); C_n2 = X*conj(roll(X,n2)) + prev-frame; U_n2 = circulant(w[:,n2]);
  V_n2 = U_n2 * roll(X,n2); delta = sum_n2 G @ V_n2; out = x + ISTFT(delta)*P (+bias)

8 cores = (b x m x n2-half), uniform SPMD program; per-core variation only in
input data (permutation stack S, circulant stack M).  Device layout: [freq(80)
partitions, time free].  STFT fp32, rest bf16 (PSUM fp32).

v5: 4 chunks of 5 n2-planes, software-pipelined across engines.  All 2-input
TENSOR_TENSOR on Vector only (GpSimd shares Vector's 2nd SBUF port and fully
blocks it); Scalar does PSUM evictions; V=U*R complex combine is folded into
+-G weight variants with PSUM accumulation; chunk pairs share one D PSUM bank;
the 5-plane D reduction is one vector tensor_reduce over a stride-permuted
view; the device returns raw delta frames [F, 2T] and the host does the ISTFT
overlap-add/scaling (numpy).
"""

import numpy as np
from ml_dtypes import bfloat16

import concourse.bass as bass
import concourse.bacc as bacc
import concourse.mybir as mybir
import concourse.tile as tile

F = 80
T = 51
TP = 52          # padded slot stride
HOP = 40
L = 2080
NJ = 20
NCH = 4
CHJ = NJ // NCH  # 5 planes per chunk
BL = CHJ * TP    # 260
FP32 = mybir.dt.float32
BF16 = mybir.dt.bfloat16

N2_LISTS = [list(range(19, -1, -1)), list(range(-1, -21, -1))]


def _dft_consts():
    j = np.arange(F)
    W = np.exp(-2j * np.pi * np.outer(j, j) / F)
    G = np.exp(+2j * np.pi * np.outer(j, j) / F) / F
    return W, G


def build_program(debug=False):
    nc = bacc.Bacc("TRN2", target_bir_lowering=False, debug=debug)

    # xf = [fiN | fr | fi] frames, pre-framed on host (pure reshape)
    xf = nc.dram_tensor("xf", [F, 3 * T], BF16, kind="ExternalInput")
    fr_c = nc.dram_tensor("fr_c", [F, 2 * F], BF16, kind="ExternalInput")
    gr_c = nc.dram_tensor("gr_c", [F, 4 * F], BF16, kind="ExternalInput")
    smat = nc.dram_tensor("smat", [F, NJ * F], BF16, kind="ExternalInput")
    mst = nc.dram_tensor("mst", [F, NJ * 2 * F], BF16, kind="ExternalInput")
    yv = nc.dram_tensor("yv", [F, 2 * T], FP32, kind="ExternalOutput")

    with tile.TileContext(nc) as tc:
        with (
            tc.tile_pool(name="const", bufs=1) as cpool,
            tc.tile_pool(name="work", bufs=1) as wpool,
            tc.tile_pool(name="ps_x", bufs=1, space="PSUM") as ps_x,
            tc.tile_pool(name="ps_r", bufs=2, space="PSUM") as ps_r,
            tc.tile_pool(name="ps_u", bufs=3, space="PSUM") as ps_u,
            tc.tile_pool(name="ps_d", bufs=1, space="PSUM") as ps_d,
        ):
            TT = nc.vector.tensor_tensor
            MUL = mybir.AluOpType.mult
            ADD = mybir.AluOpType.add
            SUB = mybir.AluOpType.subtract
            CPY = mybir.ActivationFunctionType.Copy

            # ---- preamble: DMAs chunk-split across the 3 rings so the first
            # consumers unblock earliest (smat chunk c feeds R(c))
            Xsb = wpool.tile([F, 2 * TP], BF16, tag="Xsb")
            nc.gpsimd.memset(Xsb[:, :], 0)
            frm = wpool.tile([F, 3 * T], BF16, tag="frm")
            nc.sync.dma_start(frm[:, :], xf[:, :])
            Fc = cpool.tile([F, 2 * F], BF16, tag="Fc")
            nc.sync.dma_start(Fc[:, :], fr_c[:, :])
            Ssb = cpool.tile([F, NJ * F], BF16, tag="Ssb")
            Q = NJ * F // 4
            nc.sync.dma_start(Ssb[:, 0:2 * Q], smat[:, 0:2 * Q])
            nc.scalar.dma_start(Ssb[:, 2 * Q:3 * Q], smat[:, 2 * Q:3 * Q])
            nc.gpsimd.dma_start(Ssb[:, 3 * Q:4 * Q], smat[:, 3 * Q:4 * Q])
            Msb = cpool.tile([F, NJ * 2 * F], BF16, tag="Msb")
            M4 = NJ * 2 * F // 4
            nc.scalar.dma_start(Msb[:, 0:M4], mst[:, 0:M4])
            nc.gpsimd.dma_start(Msb[:, M4:2 * M4], mst[:, M4:2 * M4])
            nc.sync.dma_start(Msb[:, 2 * M4:], mst[:, 2 * M4:])
            Gc = cpool.tile([F, 4 * F], BF16, tag="Gc")
            nc.gpsimd.dma_start(Gc[:, :], gr_c[:, :])

            # ---- STFT (fp32) -> X bf16 [Xr(52) | Xi(52)] ----
            Xp = ps_x.tile([F, 2 * T], FP32, tag="Xp")
            nc.tensor.matmul(Xp[:, :], Fc[:, 0:F], frm[:, T:3 * T], start=True, stop=False)
            nc.tensor.matmul(Xp[:, :], Fc[:, F:2 * F], frm[:, 0:2 * T], start=False, stop=True)
            Xsv = Xsb[:, :].rearrange("p (c t) -> p c t", c=2)
            nc.scalar.activation(Xsv[:, :, 0:T],
                                 Xp[:, :].rearrange("p (c t) -> p c t", c=2),
                                 CPY)
            Xrhs = bass.AP(tensor=Xsb[:, :].tensor, offset=Xsb[:, :].offset,
                           ap=[[2 * TP, F], [TP, 2], [1, T]])
            Xbr = Xsb[:, None, 0:TP].to_broadcast([F, CHJ, TP])
            Xbi = Xsb[:, None, TP:2 * TP].to_broadcast([F, CHJ, TP])

            # ---- per-chunk tiles ----
            Rsb = [wpool.tile([F, 2 * BL], BF16, tag=f"Rsb{c}", name=f"Rsb{c}")
                   for c in range(NCH)]
            Cc_ = [wpool.tile([F, 2 * BL], BF16, tag=f"Cc{c}", name=f"Cc{c}")
                   for c in range(NCH)]
            Cp_ = [wpool.tile([F, 3 * BL], BF16, tag=f"Cp{c}", name=f"Cp{c}")
                   for c in range(NCH)]
            Uc_ = [wpool.tile([F, 2 * BL], BF16, tag=f"Uc{c}", name=f"Uc{c}")
                   for c in range(NCH)]
            s4_ = [[wpool.tile([F, BL], BF16, tag=f"s{k}{c}", name=f"s{k}{c}")
                    for k in range(4)] for c in range(NCH)]
            v4_ = [[wpool.tile([F, BL], BF16, tag=f"v{k}{c}", name=f"v{k}{c}")
                    for k in range(4)] for c in range(NCH)]
            Dc_ = [wpool.tile([F, 2 * T], FP32, tag=f"Dc{p}", name=f"Dc{p}")
                   for p in range(2)]
            Dp_ = [None] * 2

            # ---- emission helpers (each touches a single engine stream) ----
            def emit_R(c):          # PE: 5 perm matmuls -> ps_r bank
                # 512 (full bank) so the 52-wide evict read stays in bounds
                Rp = ps_r.tile([F, 512], FP32, tag="Rp")
                for s in range(CHJ):
                    j = c * CHJ + s
                    nc.tensor.matmul(Rp[:, s * 2 * T:(s + 1) * 2 * T],
                                     Ssb[:, j * F:(j + 1) * F],
                                     Xrhs, start=True, stop=True)
                return Rp

            def _evict(dst_tile, Pp):
                # psum [s, c2, t] -> [c2-block, s*TP + t]
                dst = bass.AP(tensor=dst_tile[:, :].tensor,
                              offset=dst_tile[:, :].offset,
                              ap=[[2 * BL, F], [TP, CHJ], [BL, 2], [1, T]])
                src = bass.AP(tensor=Pp[:, :].tensor, offset=Pp[:, :].offset,
                              ap=[[512, F], [2 * T, CHJ], [T, 2], [1, T]])
                nc.scalar.activation(dst, src, CPY)

            def emit_Rev(c, Rp):    # scalar: evict bank -> Rsb[c] plane-major
                _evict(Rsb[c], Rp)

            def emit_P(c):          # vector: 6 products + troll + wrap
                Rr = Rsb[c][:, 0:BL]
                Ri = Rsb[c][:, BL:2 * BL]
                Rr3 = Rr.rearrange("p (j t) -> p j t", j=CHJ)
                Ri3 = Ri.rearrange("p (j t) -> p j t", j=CHJ)
                sA, sB, sC, sD = s4_[c]
                Cc = Cc_[c]
                TT(sA[:, :].rearrange("p (j t) -> p j t", j=CHJ), Xbr, Rr3, MUL)
                TT(sB[:, :].rearrange("p (j t) -> p j t", j=CHJ), Xbi, Ri3, MUL)
                TT(Cc[:, 0:BL], sA[:, :], sB[:, :], ADD)
                TT(sC[:, :].rearrange("p (j t) -> p j t", j=CHJ), Xbi, Rr3, MUL)
                TT(sD[:, :].rearrange("p (j t) -> p j t", j=CHJ), Xbr, Ri3, MUL)
                TT(Cc[:, BL:2 * BL], sC[:, :], sD[:, :], SUB)
                # C' = C + roll_t(C) into Cp = [CiN' | Cr' | Ci']
                Cp = Cp_[c]
                d_in = bass.AP(tensor=Cc[:, :].tensor, offset=Cc[:, :].offset,
                               ap=[[2 * BL, F], [BL, 2], [TP, CHJ], [1, T - 1]])
                d_in1 = bass.AP(tensor=Cc[:, :].tensor, offset=Cc[:, :].offset + 1,
                                ap=[[2 * BL, F], [BL, 2], [TP, CHJ], [1, T - 1]])
                d_out = bass.AP(tensor=Cp[:, :].tensor, offset=Cp[:, :].offset + BL + 1,
                                ap=[[3 * BL, F], [BL, 2], [TP, CHJ], [1, T - 1]])
                TT(d_out, d_in1, d_in, ADD)
                w_in = bass.AP(tensor=Cc[:, :].tensor, offset=Cc[:, :].offset,
                               ap=[[2 * BL, F], [BL, 2], [TP, CHJ], [1, 1]])
                w_in1 = bass.AP(tensor=Cc[:, :].tensor, offset=Cc[:, :].offset + T - 1,
                                ap=[[2 * BL, F], [BL, 2], [TP, CHJ], [1, 1]])
                w_out = bass.AP(tensor=Cp[:, :].tensor, offset=Cp[:, :].offset + BL,
                                ap=[[3 * BL, F], [BL, 2], [TP, CHJ], [1, 1]])
                TT(w_out, w_in, w_in1, ADD)

            def emit_CiN(c):        # vector: CiN' = -Ci' (single-src 4x mode)
                Cp = Cp_[c]
                nc.vector.tensor_scalar_mul(Cp[:, 0:BL], Cp[:, 2 * BL:3 * BL], -1.0)

            def emit_S1(c):         # PE: 10 matmuls -> ps_u bank
                Cp = Cp_[c]
                Up = ps_u.tile([F, 512], FP32, tag="Up")
                for s in range(CHJ):
                    j = c * CHJ + s
                    rhs1 = bass.AP(tensor=Cp[:, :].tensor,
                                   offset=Cp[:, :].offset + BL + s * TP,
                                   ap=[[3 * BL, F], [BL, 2], [1, T]])
                    rhs2 = bass.AP(tensor=Cp[:, :].tensor,
                                   offset=Cp[:, :].offset + s * TP,
                                   ap=[[3 * BL, F], [BL, 2], [1, T]])
                    nc.tensor.matmul(Up[:, s * 2 * T:(s + 1) * 2 * T],
                                     Msb[:, (2 * j) * F:(2 * j + 1) * F],
                                     rhs1, start=True, stop=False)
                    nc.tensor.matmul(Up[:, s * 2 * T:(s + 1) * 2 * T],
                                     Msb[:, (2 * j + 1) * F:(2 * j + 2) * F],
                                     rhs2, start=False, stop=True)
                return Up

            def emit_Uev(c, Up):    # scalar: evict bank -> Uc[c]
                _evict(Uc_[c], Up)

            def emit_V(c):          # vector: 4 products
                Rr = Rsb[c][:, 0:BL]
                Ri = Rsb[c][:, BL:2 * BL]
                Ur = Uc_[c][:, 0:BL]
                Ui = Uc_[c][:, BL:2 * BL]
                vA, vB, vC, vD = v4_[c]
                TT(vA[:, :], Ur, Rr, MUL)
                TT(vB[:, :], Ui, Ri, MUL)
                TT(vC[:, :], Ur, Ri, MUL)
                TT(vD[:, :], Ui, Rr, MUL)

            def emit_G(c):          # PE: 8 matmuls; chunk pair (2p, 2p+1)
                # accumulates into one shared ps_d bank
                vA, vB, vC, vD = v4_[c]
                p = c // 2
                if c % 2 == 0:
                    Dp_[p] = ps_d.tile([F, CHJ * 2 * T], FP32, tag=f"Dp{p}",
                                       name=f"Dp{p}")
                Dp = Dp_[p]
                Gr = Gc[:, 0:F]
                Gi = Gc[:, F:2 * F]
                Grn = Gc[:, 2 * F:3 * F]
                Gin = Gc[:, 3 * F:4 * F]

                def vrhs(tile_):
                    return bass.AP(tensor=tile_[:, :].tensor,
                                   offset=tile_[:, :].offset,
                                   ap=[[BL, F], [TP, CHJ], [1, T]])

                def dout(c2):
                    return bass.AP(tensor=Dp[:, :].tensor,
                                   offset=Dp[:, :].offset + c2 * T,
                                   ap=[[CHJ * 2 * T, F], [2 * T, CHJ], [1, T]])

                calls = [
                    (0, Gr, vA), (0, Grn, vB), (0, Gin, vC), (0, Gin, vD),
                    (1, Gr, vC), (1, Gr, vD), (1, Gi, vA), (1, Gin, vB),
                ]
                # one start (first call of even chunk) and one stop (last call
                # of odd chunk) per bank: start clears the whole bank
                for i, (c2, w, v) in enumerate(calls):
                    nc.tensor.matmul(dout(c2), w, vrhs(v),
                                     start=(c % 2 == 0 and i == 0),
                                     stop=(c % 2 == 1 and i == len(calls) - 1))

            def emit_red(p):        # vector: 5-plane reduce -> Dc[p]
                Dp = Dp_[p]
                rin = bass.AP(tensor=Dp[:, :].tensor, offset=Dp[:, :].offset,
                              ap=[[CHJ * 2 * T, F], [1, 2 * T], [2 * T, CHJ]])
                nc.vector.tensor_reduce(
                    Dc_[p][:, :].rearrange("p (ct one) -> p ct one", one=1),
                    rin, axis=mybir.AxisListType.X, op=ADD)

            # ---- pipelined emission ----
            Rps = [emit_R(c) for c in range(NCH)]
            emit_Rev(0, Rps[0])
            emit_Rev(1, Rps[1])
            emit_P(0)
            emit_CiN(0)
            Up0 = emit_S1(0)
            emit_Rev(2, Rps[2])
            emit_P(1)
            emit_CiN(1)
            emit_Uev(0, Up0)
            Up1 = emit_S1(1)
            emit_Rev(3, Rps[3])
            emit_P(2)
            emit_CiN(2)
            Up2 = emit_S1(2)
            emit_V(0)
            emit_G(0)
            emit_Uev(1, Up1)
            emit_P(3)
            emit_CiN(3)
            Up3 = emit_S1(3)
            emit_V(1)
            emit_G(1)
            emit_Uev(2, Up2)
            emit_V(2)
            emit_G(2)
            emit_red(0)
            emit_Uev(3, Up3)
            emit_V(3)
            emit_G(3)
            emit_red(1)

            # ---- final: Dsum = Dc0 + Dc1, DMA raw delta frames to host ----
            Dsb = wpool.tile([F, 2 * T], FP32, tag="Dsb")
            TT(Dsb[:, :], Dc_[0][:, :], Dc_[1][:, :], ADD)
            nc.sync.dma_start(yv[:, :], Dsb[:, :])
    return nc


# ---------------- host side ----------------

def _host_consts():
    W, G = _dft_consts()
    fr_c = np.concatenate([W.real, W.imag], axis=1).astype(bfloat16)
    gr_c = np.concatenate([G.real, G.imag, -G.real, -G.imag], axis=1).astype(bfloat16)
    cov = np.zeros(L)
    idx = (np.arange(T)[:, None] * HOP + np.arange(F)[None, :]).reshape(-1)
    np.add.at(cov, idx, 1.0)
    cov = np.where(cov > 0, cov, 1.0)
    return fr_c, gr_c, cov


def _smat_for(n2_list):
    S = np.zeros((NJ, F, F), np.float32)
    g = np.arange(F)
    for j, n2 in enumerate(n2_list):
        S[j, (g - n2) % F, g] = 1.0
    return np.ascontiguousarray(S.transpose(1, 0, 2).reshape(F, NJ * F)).astype(bfloat16)


def _mst_for(n2_list, w2):
    Ms = np.zeros((NJ, 2, F, F), np.float32)
    g = np.arange(F)[:, None]
    f = np.arange(F)[None, :]
    n1 = ((f - g + 20) % F) - 20
    valid = (n1 >= -20) & (n1 <= 19)
    n1c = np.clip(n1 + 20, 0, 39)
    for j, n2 in enumerate(n2_list):
        col = w2[:, n2 + 20]
        Ms[j, 0] = np.where(valid, col.real[n1c], 0.0)
        Ms[j, 1] = np.where(valid, col.imag[n1c], 0.0)
    return np.ascontiguousarray(
        Ms.transpose(2, 0, 1, 3).reshape(F, NJ * 2 * F)).astype(bfloat16)


def _frame(sig):
    idx = np.arange(T)[None, :] * HOP + np.arange(F)[:, None]   # [j, t]
    return sig[idx].astype(np.float32)


def make_in_maps(x_real, x_imag, task_info, w_real, w_imag):
    fr_c, gr_c, cov = _host_consts()
    b, _, m = x_real.shape
    P = np.power(10.0, task_info[:, 0] / 10.0) / m
    w2 = (np.asarray(w_real) + 1j * np.asarray(w_imag)).reshape(40, 40)
    smats = [_smat_for(nl) for nl in N2_LISTS]
    msts = [_mst_for(nl, w2) for nl in N2_LISTS]

    in_maps, shards = [], []
    for bb in range(b):
        for mm in range(m):
            fr_ = _frame(x_real[bb, :, mm])
            fi_ = _frame(x_imag[bb, :, mm])
            xfv = np.concatenate([-fi_, fr_, fi_], axis=1).astype(bfloat16)
            for h in range(2):
                in_maps.append({
                    "xf": xfv,
                    "fr_c": fr_c,
                    "gr_c": gr_c,
                    "smat": smats[h],
                    "mst": msts[h],
                })
                shards.append((bb, mm, h))
    return in_maps, shards, P, cov


_NC_CACHE = {}


def kernel(x_real, x_imag, task_info, w_real, w_imag, b_real, b_imag):
    x_real = np.asarray(x_real)
    x_imag = np.asarray(x_imag)
    task_info = np.asarray(task_info)
    b, Lx, m = x_real.shape
    assert (b, Lx, m) == (2, L, 2)

    if "nc" not in _NC_CACHE:
        nc_ = build_program(debug=False)
        nc_.compile()
        _NC_CACHE["nc"] = nc_
    nc = _NC_CACHE["nc"]

    in_maps, shards, P, cov = make_in_maps(x_real, x_imag, task_info, w_real, w_imag)
    from concourse.bass_utils import run_bass_kernel_spmd
    res = run_bass_kernel_spmd(nc, in_maps, list(range(8))).results

    x = (x_real + 1j * x_imag).astype(np.complex64)
    out = x.copy()
    bias = complex(np.asarray(b_real)[0], np.asarray(b_imag)[0])
    for i, (bb, mm, h) in enumerate(shards):
        dsum = res[i]["yv"]                 # [80, 102] = [f-sample, (dr | di)]
        fr_frames = dsum[:, 0:T] + 1j * dsum[:, T:2 * T]   # [80, T]
        yb = np.zeros((52, HOP), np.complex64)
        yb[0:T] += fr_frames[0:HOP].T
        yb[1:T + 1] += fr_frames[HOP:F].T
        out[bb, :, mm] += (P[bb] / cov) * yb.reshape(-1)
    bias_sig = np.zeros(L, np.complex64)
    bias_sig[np.arange(T) * HOP] = bias
    bias_sig /= cov
    for bb in range(b):
        for mm in range(m):
            out[bb, :, mm] += (P[bb] * bias_sig).astype(np.complex64)
    return out[:, 20:L - 20, :]
